# revision 32
# baseline (speedup 1.0000x reference)
"""Single-head causal attention kernel for Trainium2 (8 NeuronCores).

Problem: x[8, 2048, 1024], Wq/Wk/Wv[1024, 64] ->
  out[b] = softmax(causal((x[b] @ Wq) @ (x[b] @ Wk)^T / 8)) @ (x[b] @ Wv)

Sharding: data-parallel over batch, one batch element per core, weights
replicated.

v2 design (vs v1 baseline at 62.8us):
  - x and weights are converted to bf16 on the host (matmul rate is the same
    as f32r but DMA bytes halve; accumulation stays f32 in PSUM; measured
    rel-err ~2e-3 vs the 2e-2 gate)
  - host pre-packs [Wq|Wk|Wv] into one [128, 8, 192] chunk-major tensor so
    the weight load is a single 128x3KB-descriptor DMA (v1 used 256B
    descriptors which pay a 2x small-transfer penalty)
  - x is loaded with 8 large DMAs (block-major), front-loaded in consumption
    order; block 0 is split into 2-chunk pieces so proj(0) can start early
  - per t-block: q chain [64,512] + [Wk|Wv] chain [128,512]; k and v land in
    one [128,T] sbuf tile with a single PSUM->SBUF copy (k rows 0:64 base 0
    for the score matmul, v rows 64:128 base 64 for the PE transpose)
  - scores computed transposed: ST_j = kT_j^T . qT, diag tile masked
    additively, exp on the scalar engine into bf16 PT
  - P@V is split into four 128-column accumulation groups per block, so the
    contraction skips chunks with j > 4b+g (17408 PE rows instead of 20480)
    and no pt zero-padding is needed
  - softmax denominator comes from an appended ones column in v (row 64 of
    the PV output); normalization = DVE reciprocal + Pool-engine
    partition_broadcast + DVE multiply (v1 used a PE broadcast matmul)
"""

import numpy as np
from contextlib import ExitStack

import concourse.bass as bass
import concourse.tile as tile
import concourse.bacc as bacc
from concourse import mybir
from concourse import bass_utils
from concourse.masks import make_identity

F32 = mybir.dt.float32
BF16 = mybir.dt.bfloat16

T = 2048
C = 1024
H = 64
NCH = C // 128   # 8 contraction chunks
NB = T // 512    # 4 t-blocks
NEG = -1.0e9


def build_bass():
    nc = bacc.Bacc("TRN2", target_bir_lowering=False, debug=False, num_devices=8)
    xT = nc.dram_tensor("xT", [C, T], BF16, kind="ExternalInput").ap()
    wqkv = nc.dram_tensor("wqkv", [128, NCH, 192], BF16, kind="ExternalInput").ap()
    outT = nc.dram_tensor("outT", [H, T], F32, kind="ExternalOutput").ap()

    with tile.TileContext(nc) as tc:
        with ExitStack() as ctx:
            build_kernel(ctx, tc, nc, xT, wqkv, outT)
    nc.compile()
    return nc


def build_kernel(ctx, tc, nc, xT, wqkv, outT):
    const = ctx.enter_context(tc.tile_pool(name="const", bufs=1))
    pt_pool = ctx.enter_context(tc.tile_pool(name="pt", bufs=4))
    fin_pool = ctx.enter_context(tc.tile_pool(name="fin", bufs=2))
    rc_pool = ctx.enter_context(tc.tile_pool(name="rc", bufs=2))
    rb_pool = ctx.enter_context(tc.tile_pool(name="rb", bufs=2))
    qk_ps = ctx.enter_context(tc.tile_pool(name="qkps", bufs=1, space="PSUM"))
    kv_ps = ctx.enter_context(tc.tile_pool(name="kvps", bufs=1, space="PSUM"))
    st_ps = ctx.enter_context(tc.tile_pool(name="stps", bufs=2, space="PSUM"))
    o_ps = ctx.enter_context(tc.tile_pool(name="ops", bufs=1, space="PSUM"))

    # persistent sbuf state
    xt = const.tile([128, NB, NCH, 512], BF16)   # x^T, block-major chunks
    w = const.tile([128, NCH, 192], BF16)        # [Wq|Wk|Wv] per c-chunk
    qT_sb = const.tile([64, T], BF16)
    kv_sb = const.tile([128, T], BF16)           # rows 0:64 kT, 64:128 vT
    v_sb = const.tile([128, T // 128, H + 1], BF16)  # v natural + ones col
    neg_mask = const.tile([128, 128], F32)       # 0 where t>=s, -1e9 below
    idb = const.tile([128, 64], BF16)            # identity in rows 64:128

    # weights then x, in consumption order; block 0 in small pieces so the
    # first projection chain can start as soon as possible
    nc.sync.dma_start(w, wqkv)
    xr = xT.rearrange("(j p) t -> p j t", p=128)
    for i in range(4):
        nc.sync.dma_start(xt[:, 0, 2 * i : 2 * i + 2, :],
                          xr[:, 2 * i : 2 * i + 2, 0:512])
    for b in (1, 2, 3):
        blk = slice(512 * b, 512 * (b + 1))
        nc.sync.dma_start(xt[:, b, 0:4, :], xr[:, 0:4, blk])
        nc.sync.dma_start(xt[:, b, 4:8, :], xr[:, 4:8, blk])

    # constants
    nc.gpsimd.memset(neg_mask, 0.0)
    nc.gpsimd.affine_select(
        out=neg_mask, in_=neg_mask, compare_op=mybir.AluOpType.is_ge,
        fill=NEG, base=0, pattern=[[1, 128]], channel_multiplier=-1,
    )
    id_f = const.tile([128, 64], F32)
    make_identity(nc, id_f[64:128, :])
    nc.vector.tensor_copy(idb[64:128, :], id_f[64:128, :])
    ones_f = const.tile([128, 16, 1], F32)
    nc.vector.memset(ones_f, 1.0)
    nc.vector.tensor_copy(v_sb[:, :, H : H + 1], ones_f)
    # dedicated PT slots for diagonal s-chunks, one per within-block offset r:
    # the pad region [0:128r] is zeroed once here and never overwritten (exp
    # always writes exactly [128r:512]), so the full-width P@V matmul reads
    # zeros above the diagonal
    pt_diag = {r: const.tile([128, 512], BF16, name=f"pt_diag{r}")
               for r in range(1, 4)}
    zero_f = const.tile([128, 384], F32)
    nc.vector.memset(zero_f, 0.0)
    for r in range(1, 4):
        nc.vector.tensor_copy(pt_diag[r][:, 0 : 128 * r], zero_f[:, 0 : 128 * r])

    def gen_proj(b):
        """Projection for block b as a generator of 9 'pieces' so it can be
        interleaved into the previous block's attention chunk loop (keeps the
        PE fed while the scalar engine works through the exps)."""
        blk = slice(512 * b, 512 * (b + 1))
        q_t = qk_ps.tile([64, 512], F32, tag="q")
        kv_t = kv_ps.tile([128, 512], F32, tag="kv")
        for j in range(NCH):
            nc.tensor.matmul(q_t, w[:, j, 0:64], xt[:, b, j, :],
                             start=(j == 0), stop=(j == NCH - 1))
            nc.tensor.matmul(kv_t, w[:, j, 64:192], xt[:, b, j, :],
                             start=(j == 0), stop=(j == NCH - 1))
            yield
        nc.vector.tensor_copy(qT_sb[:, blk], q_t)
        nc.vector.tensor_copy(kv_sb[:, blk], kv_t)
        # v natural layout via PE transpose; vT sits at rows 64:128 so the
        # transpose operands (vT slice, identity rows 64:128) share
        # base_partition
        tp = qk_ps.tile([128, 4, 64], BF16, tag="q", name=f"tp{b}")
        for r in range(4):
            nc.tensor.transpose(
                tp[:, r, :],
                kv_sb[64:128, 512 * b + 128 * r : 512 * b + 128 * (r + 1)],
                idb[64:128, :])
        nc.vector.tensor_copy(v_sb[:, 4 * b : 4 * b + 4, 0:H], tp)
        yield

    def att(b, nxt=None):
        blk = slice(512 * b, 512 * (b + 1))
        # P@V accumulates into two independent 256-column groups living in
        # separate PSUM banks (full-bank-shaped tiles force the separation;
        # two accumulation groups must never share a bank). Group 0 covers
        # cols 0:256 and stops at chunk 4b+1, so its normalization chain runs
        # ~2 chunks early and only group 1's tail is exposed at the end.
        out_g = [o_ps.tile([65, 512], F32, tag="oa", name=f"oa{b}"),
                 o_ps.tile([65, 512], F32, tag="ob", name=f"ob{b}")]
        fin = fin_pool.tile([64, 512], F32)
        nj = 4 * b + 4
        pts = {}
        _dbg_pts = []

        def tail_group(g):
            w0, w1 = (0, 384) if g == 0 else (384, 512)
            gw = w1 - w0
            rc = rc_pool.tile([1, 384], F32, tag="rc", name=f"rc{b}_{g}")
            nc.vector.reciprocal(rc[:, 0:gw], out_g[g][64:65, 0:gw])
            rb = rb_pool.tile([64, 384], F32, tag="rb", name=f"rb{b}_{g}")
            nc.gpsimd.partition_broadcast(rb[:, 0:gw], rc[:, 0:gw], channels=64)
            nc.vector.tensor_mul(fin[:, w0:w1], out_g[g][0:64, 0:gw],
                                 rb[:, 0:gw])
            nc.sync.dma_start(outT[:, 512 * b + w0 : 512 * b + w1],
                              fin[:, w0:w1])

        # chunk "units": off-diagonal chunks are paired so each pair shares
        # one [128,1024] ST tile (two PSUM banks, one accumulation group per
        # bank) and a SINGLE exp instruction — the scalar engine's ~190ns
        # fixed cost per activation is what paces the attention phase.
        # Diagonal chunks stay single (their exp widths shrink with r and the
        # zero pads in pt_diag must not be overwritten).
        units = [(2 * u, 2 * u + 1) for u in range(2 * b)]
        units += [(4 * b + r,) for r in range(4)]

        def emit_st_unit(u):
            chunks = units[u]
            j0 = chunks[0]
            r = j0 - 4 * b
            st = st_ps.tile([128, 1024], F32, tag="st", name=f"st{b}_{u}")
            if r < 0:
                pt = pt_pool.tile([128, 1024], BF16, tag="pt", name=f"pt{b}_{u}")
                for i, j in enumerate(chunks):
                    nc.tensor.matmul(
                        st[:, 512 * i : 512 * i + 512],
                        kv_sb[0:64, 128 * j : 128 * (j + 1)],
                        qT_sb[:, blk], start=True, stop=True)
                nc.scalar.activation(
                    pt, st, func=mybir.ActivationFunctionType.Exp, scale=0.125)
                for i, j in enumerate(chunks):
                    pts[j] = (pt, 512 * i)
                _dbg_pts.append(pt[:, 0:512])
                return
            else:
                coff = 128 * r
                width = 512 - coff
                if r > 0:
                    pt = pt_diag[r]
                else:
                    pt = pt_pool.tile([128, 1024], BF16, tag="pt",
                                      name=f"pt{b}_{u}")
                nc.tensor.matmul(st[:, 0:width],
                                 kv_sb[0:64, 128 * j0 : 128 * (j0 + 1)],
                                 qT_sb[:, 512 * b + coff : 512 * (b + 1)],
                                 start=True, stop=True)
                nc.vector.tensor_add(st[:, 0:128], st[:, 0:128], neg_mask)
                nc.scalar.activation(
                    pt[:, coff:512], st[:, 0:width],
                    func=mybir.ActivationFunctionType.Exp, scale=0.125)
                pts[j0] = (pt, 0)
            _dbg_pts.append(pt[:, 0:512])

        def emit_pv_unit(u):
            for j in units[u]:
                r = j - 4 * b
                pt, f = pts.pop(j)
                for g, w0, w1, jstop in ((0, 0, 384, 4 * b + 2),
                                         (1, 384, 512, 4 * b + 3)):
                    if (g == 0 and r > 2) or j > jstop:
                        continue
                    nc.tensor.matmul(out_g[g][:, 0 : w1 - w0], v_sb[:, j, :],
                                     pt[:, f + w0 : f + w1],
                                     start=(j == 0), stop=(j == jstop))
                    if j == jstop:
                        tail_group(g)

        # software-pipeline the emission so the PE never head-blocks on an
        # exp that hasn't finished (PV for unit u emitted after ST of unit
        # u+2), and drain the next block's projection pieces into the unit
        # slots; the drain is skewed toward later slots because the x DMA for
        # block b+1 is still in flight during the early slots
        NPC = 9
        nu = len(units)
        drained = 0
        for u in range(nu):
            emit_st_unit(u)
            if nxt is not None:
                frac = ((u + 1) / nu) ** 1.5
                target = min(NPC, int(NPC * frac + 0.999))
                while drained < target:
                    next(nxt, None)
                    drained += 1
            if u >= 2:
                emit_pv_unit(u - 2)
        if nxt is not None:
            while drained < NPC:
                next(nxt, None)
                drained += 1
        for u in range(max(nu - 2, 0), nu):
            emit_pv_unit(u)

        if DEBUG_ATT and b == 0:
            dbg = const.tile([65, 512], F32)
            nc.vector.tensor_copy(dbg[:, 0:384], out_g[0][:, 0:384])
            nc.vector.tensor_copy(dbg[:, 384:512], out_g[1][:, 0:128])
            _DBG["out_t0"] = dbg
            for j, p in enumerate(_dbg_pts):
                nc.sync.dma_start(_DBG["d_pt_aps"][j], p)

    gens = [gen_proj(b) for b in range(NB)]
    for _ in gens[0]:
        pass
    for b in range(NB):
        att(b, gens[b + 1] if b + 1 < NB else None)

    # debug hook: stash persistent tiles so a debug build can dump them
    _DBG.update({"qT_sb": qT_sb, "kv_sb": kv_sb, "v_sb": v_sb, "xt": xt,
                 "w": w})


_DBG = {}
DEBUG_ATT = False


_NC = None


def _get_nc():
    global _NC
    if _NC is None:
        _NC = build_bass()
    return _NC


def _pack_w(Wq, Wk, Wv, npbf):
    def chunks(W):
        return np.ascontiguousarray(W.reshape(NCH, 128, H).transpose(1, 0, 2))
    return np.ascontiguousarray(
        np.concatenate([chunks(Wq), chunks(Wk), chunks(Wv)], axis=2)
    ).astype(npbf)


def kernel(x, Wq, Wk, Wv):
    nc = _get_nc()
    npbf = mybir.dt.np(BF16)
    wqkv = _pack_w(Wq, Wk, Wv, npbf)
    in_maps = []
    for b in range(8):
        in_maps.append({
            "xT": np.ascontiguousarray(x[b].T).astype(npbf),
            "wqkv": wqkv,
        })
    res = bass_utils.run_bass_kernel_spmd(nc, in_maps, core_ids=list(range(8)))
    out = np.stack([np.ascontiguousarray(res.results[b]["outT"].T)
                    for b in range(8)])
    return out.astype(np.float32)


# revision 49
# speedup vs baseline: 1.0720x; 1.0720x over previous
"""Single-head causal attention kernel for Trainium2 (8 NeuronCores).

Problem: x[8, 2048, 1024], Wq/Wk/Wv[1024, 64] ->
  out[b] = softmax(causal((x[b] @ Wq) @ (x[b] @ Wk)^T / 8)) @ (x[b] @ Wv)

Sharding: data-parallel over batch, one batch element per core, weights
replicated.

v2 design (vs v1 baseline at 62.8us):
  - x and weights are converted to bf16 on the host (matmul rate is the same
    as f32r but DMA bytes halve; accumulation stays f32 in PSUM; measured
    rel-err ~2e-3 vs the 2e-2 gate)
  - host pre-packs [Wq|Wk|Wv] into one [128, 8, 192] chunk-major tensor so
    the weight load is a single 128x3KB-descriptor DMA (v1 used 256B
    descriptors which pay a 2x small-transfer penalty)
  - x is loaded with 8 large DMAs (block-major), front-loaded in consumption
    order; block 0 is split into 2-chunk pieces so proj(0) can start early
  - per t-block: q chain [64,512] + [Wk|Wv] chain [128,512]; k and v land in
    one [128,T] sbuf tile with a single PSUM->SBUF copy (k rows 0:64 base 0
    for the score matmul, v rows 64:128 base 64 for the PE transpose)
  - scores computed transposed: ST_j = kT_j^T . qT, diag tile masked
    additively, exp on the scalar engine into bf16 PT
  - P@V is split into four 128-column accumulation groups per block, so the
    contraction skips chunks with j > 4b+g (17408 PE rows instead of 20480)
    and no pt zero-padding is needed
  - softmax denominator comes from an appended ones column in v (row 64 of
    the PV output); normalization = DVE reciprocal + Pool-engine
    partition_broadcast + DVE multiply (v1 used a PE broadcast matmul)
"""

import numpy as np
from contextlib import ExitStack

import concourse.bass as bass
import concourse.tile as tile
import concourse.bacc as bacc
from concourse import mybir
from concourse import bass_utils
from concourse.masks import make_identity

F32 = mybir.dt.float32
BF16 = mybir.dt.bfloat16

T = 2048
C = 1024
H = 64
NCH = C // 128   # 8 contraction chunks
NB = T // 512    # 4 t-blocks
NEG = -1.0e9

# schedule knobs (tuned against TimelineSim)
CONFIG = {
    # x DMA stream: list of (block, chunk_lo, chunk_hi) in issue order
    "x_pieces": [(0, 0, 2), (0, 2, 4), (0, 4, 6), (0, 6, 8),
                 (1, 0, 2), (1, 2, 4), (1, 4, 6), (1, 6, 8),
                 (2, 0, 8), (3, 0, 8)],
    # per-unit cumulative drain targets of the next block's 9 proj pieces,
    # keyed by number of units in the current block
    "drain": {4: [2, 4, 7, 9],
              6: [2, 4, 6, 8, 9, 9],
              8: [2, 4, 5, 6, 8, 9, 9, 9]},
    # PV emission lag in units
    "pv_lag": 3,
}


def build_bass():
    nc = bacc.Bacc("TRN2", target_bir_lowering=False, debug=False, num_devices=8)
    xT = nc.dram_tensor("xT", [C, T], BF16, kind="ExternalInput").ap()
    wq_d = nc.dram_tensor("wq", [128, NCH, 64], BF16, kind="ExternalInput").ap()
    wkv_d = nc.dram_tensor("wkv", [128, NCH, 128], BF16,
                           kind="ExternalInput").ap()
    outT = nc.dram_tensor("outT", [H, T], F32, kind="ExternalOutput").ap()

    with tile.TileContext(nc) as tc:
        with ExitStack() as ctx:
            build_kernel(ctx, tc, nc, xT, (wq_d, wkv_d), outT)
    nc.compile()
    return nc


def build_kernel(ctx, tc, nc, xT, wdrams, outT):
    wq_d, wkv_d = wdrams
    const = ctx.enter_context(tc.tile_pool(name="const", bufs=1))
    pt_pool = ctx.enter_context(
        tc.tile_pool(name="pt", bufs=CONFIG["pv_lag"] + 2))
    fin_pool = ctx.enter_context(tc.tile_pool(name="fin", bufs=2))
    rc_pool = ctx.enter_context(tc.tile_pool(name="rc", bufs=2))
    rb_pool = ctx.enter_context(tc.tile_pool(name="rb", bufs=2))
    qk_ps = ctx.enter_context(tc.tile_pool(name="qkps", bufs=1, space="PSUM"))
    kv_ps = ctx.enter_context(tc.tile_pool(name="kvps", bufs=1, space="PSUM"))
    st_ps = ctx.enter_context(tc.tile_pool(name="stps", bufs=2, space="PSUM"))
    o_ps = ctx.enter_context(tc.tile_pool(name="ops", bufs=1, space="PSUM"))

    # persistent sbuf state
    xt = const.tile([128, NB, NCH, 512], BF16)   # x^T, block-major chunks
    w_q = const.tile([128, NCH, 64], BF16)       # Wq per c-chunk
    w_kv = const.tile([128, NCH, 128], BF16)     # [Wk|Wv] per c-chunk
    qT_sb = const.tile([64, T], BF16)
    kv_sb = const.tile([128, T], BF16)           # rows 0:64 kT, 64:128 vT
    v_sb = const.tile([128, T // 128, H + 1], BF16)  # v natural + ones col
    neg_mask = const.tile([128, 128], F32)       # 0 where t>=s, -1e9 below
    idb = const.tile([128, 64], BF16)            # identity in rows 64:128

    # weights then x, in consumption order; block 0 in small pieces so the
    # first projection chain can start as soon as possible
    nc.sync.dma_start(w_q, wq_d)
    nc.sync.dma_start(w_kv, wkv_d)
    xr = xT.rearrange("(j p) t -> p j t", p=128)
    for blkno, lo, hi in CONFIG["x_pieces"]:
        blk = slice(512 * blkno, 512 * (blkno + 1))
        nc.sync.dma_start(xt[:, blkno, lo:hi, :], xr[:, lo:hi, blk])

    # constants
    nc.gpsimd.memset(neg_mask, 0.0)
    nc.gpsimd.affine_select(
        out=neg_mask, in_=neg_mask, compare_op=mybir.AluOpType.is_ge,
        fill=NEG, base=0, pattern=[[1, 128]], channel_multiplier=-1,
    )
    id_f = const.tile([128, 64], F32)
    make_identity(nc, id_f[64:128, :])
    nc.vector.tensor_copy(idb[64:128, :], id_f[64:128, :])
    ones_f = const.tile([128, 16, 1], F32)
    nc.vector.memset(ones_f, 1.0)
    nc.vector.tensor_copy(v_sb[:, :, H : H + 1], ones_f)
    # dedicated PT slots for diagonal s-chunks, one per within-block offset r:
    # the pad region [0:128r] is zeroed once here and never overwritten (exp
    # always writes exactly [128r:512]), so the full-width P@V matmul reads
    # zeros above the diagonal
    pt_diag = {r: const.tile([128, 512], BF16, name=f"pt_diag{r}")
               for r in range(1, 4)}
    zero_f = const.tile([128, 384], F32)
    nc.vector.memset(zero_f, 0.0)
    for r in range(1, 4):
        nc.vector.tensor_copy(pt_diag[r][:, 0 : 128 * r], zero_f[:, 0 : 128 * r])

    def gen_proj(b):
        """Projection for block b as a generator of 9 'pieces' so it can be
        interleaved into the previous block's attention chunk loop (keeps the
        PE fed while the scalar engine works through the exps)."""
        blk = slice(512 * b, 512 * (b + 1))
        q_t = qk_ps.tile([64, 512], F32, tag="q")
        kv_t = kv_ps.tile([128, 512], F32, tag="kv")
        for j in range(NCH):
            nc.tensor.matmul(q_t, w_q[:, j, :], xt[:, b, j, :],
                             start=(j == 0), stop=(j == NCH - 1))
            nc.tensor.matmul(kv_t, w_kv[:, j, :], xt[:, b, j, :],
                             start=(j == 0), stop=(j == NCH - 1))
            yield
        nc.vector.tensor_copy(qT_sb[:, blk], q_t)
        nc.vector.tensor_copy(kv_sb[:, blk], kv_t)
        # v natural layout via PE transpose; vT sits at rows 64:128 so the
        # transpose operands (vT slice, identity rows 64:128) share
        # base_partition
        tp = qk_ps.tile([128, 4, 64], BF16, tag="q", name=f"tp{b}")
        for r in range(4):
            nc.tensor.transpose(
                tp[:, r, :],
                kv_sb[64:128, 512 * b + 128 * r : 512 * b + 128 * (r + 1)],
                idb[64:128, :])
        nc.vector.tensor_copy(v_sb[:, 4 * b : 4 * b + 4, 0:H], tp)
        yield

    def att(b, nxt=None):
        blk = slice(512 * b, 512 * (b + 1))
        # P@V accumulates into two independent 256-column groups living in
        # separate PSUM banks (full-bank-shaped tiles force the separation;
        # two accumulation groups must never share a bank). Group 0 covers
        # cols 0:256 and stops at chunk 4b+1, so its normalization chain runs
        # ~2 chunks early and only group 1's tail is exposed at the end.
        out_g = [o_ps.tile([65, 512], F32, tag="oa", name=f"oa{b}"),
                 o_ps.tile([65, 512], F32, tag="ob", name=f"ob{b}")]
        fin = fin_pool.tile([64, 512], F32)
        nj = 4 * b + 4
        pts = {}
        _dbg_pts = []

        def tail_group(g):
            cols = slice(256 * g, 256 * (g + 1))
            rc = rc_pool.tile([1, 256], F32, tag="rc", name=f"rc{b}_{g}")
            nc.vector.reciprocal(rc, out_g[g][64:65, 0:256])
            rb = rb_pool.tile([64, 256], F32, tag="rb", name=f"rb{b}_{g}")
            nc.gpsimd.partition_broadcast(rb, rc, channels=64)
            nc.vector.tensor_mul(fin[:, cols], out_g[g][0:64, 0:256], rb)
            if g == 1:
                nc.sync.dma_start(outT[:, blk], fin)

        # chunk "units": off-diagonal chunks are paired so each pair shares
        # one [128,1024] ST tile (two PSUM banks, one accumulation group per
        # bank) and a SINGLE exp instruction — the scalar engine's ~190ns
        # fixed cost per activation is what paces the attention phase.
        # Diagonal chunks stay single (their exp widths shrink with r and the
        # zero pads in pt_diag must not be overwritten).
        units = [(2 * u, 2 * u + 1) for u in range(2 * b)]
        units += [(4 * b + r,) for r in range(4)]

        def emit_st_unit(u):
            chunks = units[u]
            j0 = chunks[0]
            r = j0 - 4 * b
            st = st_ps.tile([128, 1024], F32, tag="st", name=f"st{b}_{u}")
            if r < 0:
                pt = pt_pool.tile([128, 1024], BF16, tag="pt", name=f"pt{b}_{u}")
                for i, j in enumerate(chunks):
                    nc.tensor.matmul(
                        st[:, 512 * i : 512 * i + 512],
                        kv_sb[0:64, 128 * j : 128 * (j + 1)],
                        qT_sb[:, blk], start=True, stop=True)
                nc.scalar.activation(
                    pt, st, func=mybir.ActivationFunctionType.Exp, scale=0.125)
                for i, j in enumerate(chunks):
                    pts[j] = (pt, 512 * i)
                _dbg_pts.append(pt[:, 0:512])
                return
            else:
                coff = 128 * r
                width = 512 - coff
                if r > 0:
                    pt = pt_diag[r]
                else:
                    pt = pt_pool.tile([128, 1024], BF16, tag="pt",
                                      name=f"pt{b}_{u}")
                nc.tensor.matmul(st[:, 0:width],
                                 kv_sb[0:64, 128 * j0 : 128 * (j0 + 1)],
                                 qT_sb[:, 512 * b + coff : 512 * (b + 1)],
                                 start=True, stop=True)
                nc.vector.tensor_add(st[:, 0:128], st[:, 0:128], neg_mask)
                nc.scalar.activation(
                    pt[:, coff:512], st[:, 0:width],
                    func=mybir.ActivationFunctionType.Exp, scale=0.125)
                pts[j0] = (pt, 0)
            _dbg_pts.append(pt[:, 0:512])

        def emit_pv_unit(u):
            for j in units[u]:
                r = j - 4 * b
                pt, f = pts.pop(j)
                for g in range(2):
                    if r > 2 * g + 1:
                        continue
                    jstop = 4 * b + 2 * g + 1
                    nc.tensor.matmul(out_g[g][:, 0:256], v_sb[:, j, :],
                                     pt[:, f + 256 * g : f + 256 * (g + 1)],
                                     start=(j == 0), stop=(j == jstop))
                    if j == jstop:
                        tail_group(g)

        # software-pipeline the emission so the PE never head-blocks on an
        # exp that hasn't finished (PV for unit u emitted after ST of unit
        # u+2), and drain the next block's projection pieces into the unit
        # slots; the drain is skewed toward later slots because the x DMA for
        # block b+1 is still in flight during the early slots
        NPC = 9
        nu = len(units)
        LAG = CONFIG["pv_lag"]
        targets = CONFIG["drain"][nu] if nxt is not None else None
        drained = 0
        for u in range(nu):
            emit_st_unit(u)
            if nxt is not None:
                while drained < targets[u]:
                    next(nxt, None)
                    drained += 1
            if u >= LAG:
                emit_pv_unit(u - LAG)
        if nxt is not None:
            while drained < NPC:
                next(nxt, None)
                drained += 1
        for u in range(max(nu - LAG, 0), nu):
            emit_pv_unit(u)

        if DEBUG_ATT and b == 0:
            dbg = const.tile([65, 512], F32)
            nc.vector.tensor_copy(dbg[:, 0:256], out_g[0][:, 0:256])
            nc.vector.tensor_copy(dbg[:, 256:512], out_g[1][:, 0:256])
            _DBG["out_t0"] = dbg
            for j, p in enumerate(_dbg_pts):
                nc.sync.dma_start(_DBG["d_pt_aps"][j], p)

    gens = [gen_proj(b) for b in range(NB)]
    for _ in gens[0]:
        pass
    for b in range(NB):
        att(b, gens[b + 1] if b + 1 < NB else None)

    # debug hook: stash persistent tiles so a debug build can dump them
    _DBG.update({"qT_sb": qT_sb, "kv_sb": kv_sb, "v_sb": v_sb, "xt": xt})


_DBG = {}
DEBUG_ATT = False


_NC = None


def _get_nc():
    global _NC
    if _NC is None:
        _NC = build_bass()
    return _NC


def _pack_w(Wq, Wk, Wv, npbf):
    def chunks(W):
        return np.ascontiguousarray(W.reshape(NCH, 128, H).transpose(1, 0, 2))
    wq = np.ascontiguousarray(chunks(Wq)).astype(npbf)
    wkv = np.ascontiguousarray(
        np.concatenate([chunks(Wk), chunks(Wv)], axis=2)).astype(npbf)
    return wq, wkv


def kernel(x, Wq, Wk, Wv):
    nc = _get_nc()
    npbf = mybir.dt.np(BF16)
    wq, wkv = _pack_w(Wq, Wk, Wv, npbf)
    in_maps = []
    for b in range(8):
        in_maps.append({
            "xT": np.ascontiguousarray(x[b].T).astype(npbf),
            "wq": wq,
            "wkv": wkv,
        })
    res = bass_utils.run_bass_kernel_spmd(nc, in_maps, core_ids=list(range(8)))
    out = np.stack([np.ascontiguousarray(res.results[b]["outT"].T)
                    for b in range(8)])
    return out.astype(np.float32)


# revision 57
# speedup vs baseline: 1.0739x; 1.0017x over previous
"""Single-head causal attention kernel for Trainium2 (8 NeuronCores).

Problem: x[8, 2048, 1024], Wq/Wk/Wv[1024, 64] ->
  out[b] = softmax(causal((x[b] @ Wq) @ (x[b] @ Wk)^T / 8)) @ (x[b] @ Wv)

Sharding: data-parallel over batch, one batch element per core, weights
replicated.

v2 design (vs v1 baseline at 62.8us):
  - x and weights are converted to bf16 on the host (matmul rate is the same
    as f32r but DMA bytes halve; accumulation stays f32 in PSUM; measured
    rel-err ~2e-3 vs the 2e-2 gate)
  - host pre-packs [Wq|Wk|Wv] into one [128, 8, 192] chunk-major tensor so
    the weight load is a single 128x3KB-descriptor DMA (v1 used 256B
    descriptors which pay a 2x small-transfer penalty)
  - x is loaded with 8 large DMAs (block-major), front-loaded in consumption
    order; block 0 is split into 2-chunk pieces so proj(0) can start early
  - per t-block: q chain [64,512] + [Wk|Wv] chain [128,512]; k and v land in
    one [128,T] sbuf tile with a single PSUM->SBUF copy (k rows 0:64 base 0
    for the score matmul, v rows 64:128 base 64 for the PE transpose)
  - scores computed transposed: ST_j = kT_j^T . qT, diag tile masked
    additively, exp on the scalar engine into bf16 PT
  - P@V is split into four 128-column accumulation groups per block, so the
    contraction skips chunks with j > 4b+g (17408 PE rows instead of 20480)
    and no pt zero-padding is needed
  - softmax denominator comes from an appended ones column in v (row 64 of
    the PV output); normalization = DVE reciprocal + Pool-engine
    partition_broadcast + DVE multiply (v1 used a PE broadcast matmul)
"""

import numpy as np
from contextlib import ExitStack

import concourse.bass as bass
import concourse.tile as tile
import concourse.bacc as bacc
from concourse import mybir
from concourse import bass_utils
from concourse.masks import make_identity

F32 = mybir.dt.float32
BF16 = mybir.dt.bfloat16

T = 2048
C = 1024
H = 64
NCH = C // 128   # 8 contraction chunks
NB = T // 512    # 4 t-blocks
NEG = -1.0e9

# schedule knobs (tuned against TimelineSim)
CONFIG = {
    # x DMA stream: list of (block, chunk_lo, chunk_hi) in issue order
    "x_pieces": [(0, 0, 2), (0, 2, 4), (0, 4, 6), (0, 6, 8),
                 (1, 0, 2), (1, 2, 4), (1, 4, 6), (1, 6, 8),
                 (2, 0, 8), (3, 0, 8)],
    # per-unit cumulative drain targets of the next block's 9 proj pieces,
    # keyed by number of units in the current block
    "drain": {4: [2, 4, 7, 9],
              6: [2, 4, 6, 8, 9, 9],
              8: [2, 4, 5, 6, 8, 9, 9, 9]},
    # PV emission lag in units
    "pv_lag": 3,
    # solo steps of att(2) before att(3) interleaving begins; must be large
    # enough that proj(3) is fully drained first (drain hits 9 at unit 5)
    "s2_solo": 8,
}


def build_bass():
    nc = bacc.Bacc("TRN2", target_bir_lowering=False, debug=False, num_devices=8)
    xT = nc.dram_tensor("xT", [C, T], BF16, kind="ExternalInput").ap()
    wq_d = nc.dram_tensor("wq", [128, NCH, 64], BF16, kind="ExternalInput").ap()
    wkv_d = nc.dram_tensor("wkv", [128, NCH, 128], BF16,
                           kind="ExternalInput").ap()
    outT = nc.dram_tensor("outT", [H, T], F32, kind="ExternalOutput").ap()

    with tile.TileContext(nc) as tc:
        with ExitStack() as ctx:
            build_kernel(ctx, tc, nc, xT, (wq_d, wkv_d), outT)
    nc.compile()
    return nc


def build_kernel(ctx, tc, nc, xT, wdrams, outT):
    wq_d, wkv_d = wdrams
    const = ctx.enter_context(tc.tile_pool(name="const", bufs=1))
    pt_pool = ctx.enter_context(
        tc.tile_pool(name="pt", bufs=CONFIG["pv_lag"] + 2))
    fin_pool = ctx.enter_context(tc.tile_pool(name="fin", bufs=2))
    rc_pool = ctx.enter_context(tc.tile_pool(name="rc", bufs=2))
    rb_pool = ctx.enter_context(tc.tile_pool(name="rb", bufs=2))
    qk_ps = ctx.enter_context(tc.tile_pool(name="qkps", bufs=1, space="PSUM"))
    kv_ps = ctx.enter_context(tc.tile_pool(name="kvps", bufs=1, space="PSUM"))
    st_ps = ctx.enter_context(tc.tile_pool(name="stps", bufs=2, space="PSUM"))
    o_ps = ctx.enter_context(tc.tile_pool(name="ops", bufs=1, space="PSUM"))

    # persistent sbuf state
    xt = const.tile([128, NB, NCH, 512], BF16)   # x^T, block-major chunks
    w_q = const.tile([128, NCH, 64], BF16)       # Wq per c-chunk
    w_kv = const.tile([128, NCH, 128], BF16)     # [Wk|Wv] per c-chunk
    qT_sb = const.tile([64, T], BF16)
    kv_sb = const.tile([128, T], BF16)           # rows 0:64 kT, 64:128 vT
    v_sb = const.tile([128, T // 128, H + 1], BF16)  # v natural + ones col
    neg_mask = const.tile([128, 128], F32)       # 0 where t>=s, -1e9 below
    idb = const.tile([128, 64], BF16)            # identity in rows 64:128

    # weights then x, in consumption order; block 0 in small pieces so the
    # first projection chain can start as soon as possible
    nc.sync.dma_start(w_q, wq_d)
    nc.sync.dma_start(w_kv, wkv_d)
    xr = xT.rearrange("(j p) t -> p j t", p=128)
    for blkno, lo, hi in CONFIG["x_pieces"]:
        blk = slice(512 * blkno, 512 * (blkno + 1))
        nc.sync.dma_start(xt[:, blkno, lo:hi, :], xr[:, lo:hi, blk])

    # constants
    nc.gpsimd.memset(neg_mask, 0.0)
    nc.gpsimd.affine_select(
        out=neg_mask, in_=neg_mask, compare_op=mybir.AluOpType.is_ge,
        fill=NEG, base=0, pattern=[[1, 128]], channel_multiplier=-1,
    )
    id_f = const.tile([128, 64], F32)
    make_identity(nc, id_f[64:128, :])
    nc.vector.tensor_copy(idb[64:128, :], id_f[64:128, :])
    ones_f = const.tile([128, 16, 1], F32)
    nc.vector.memset(ones_f, 1.0)
    nc.vector.tensor_copy(v_sb[:, :, H : H + 1], ones_f)
    # dedicated PT slots for diagonal s-chunks, one per within-block offset r:
    # the pad region [0:128r] is zeroed once here and never overwritten (exp
    # always writes exactly [128r:512]), so the full-width P@V matmul reads
    # zeros above the diagonal
    # two sets so adjacent blocks can be processed interleaved without
    # write-after-read fences on the shared diag slots
    pt_diag = {(s, r): const.tile([128, 512], BF16, name=f"pt_diag{s}_{r}")
               for s in range(2) for r in range(1, 4)}
    zero_f = const.tile([128, 384], F32)
    nc.vector.memset(zero_f, 0.0)
    for s in range(2):
        for r in range(1, 4):
            nc.vector.tensor_copy(pt_diag[(s, r)][:, 0 : 128 * r],
                                  zero_f[:, 0 : 128 * r])

    def gen_proj(b):
        """Projection for block b as a generator of 9 'pieces' so it can be
        interleaved into the previous block's attention chunk loop (keeps the
        PE fed while the scalar engine works through the exps)."""
        blk = slice(512 * b, 512 * (b + 1))
        q_t = qk_ps.tile([64, 512], F32, tag="q")
        kv_t = kv_ps.tile([128, 512], F32, tag="kv")
        for j in range(NCH):
            nc.tensor.matmul(q_t, w_q[:, j, :], xt[:, b, j, :],
                             start=(j == 0), stop=(j == NCH - 1))
            nc.tensor.matmul(kv_t, w_kv[:, j, :], xt[:, b, j, :],
                             start=(j == 0), stop=(j == NCH - 1))
            yield
        nc.vector.tensor_copy(qT_sb[:, blk], q_t)
        nc.vector.tensor_copy(kv_sb[:, blk], kv_t)
        # v natural layout via PE transpose; vT sits at rows 64:128 so the
        # transpose operands (vT slice, identity rows 64:128) share
        # base_partition
        tp = qk_ps.tile([128, 4, 64], BF16, tag="q", name=f"tp{b}")
        for r in range(4):
            nc.tensor.transpose(
                tp[:, r, :],
                kv_sb[64:128, 512 * b + 128 * r : 512 * b + 128 * (r + 1)],
                idb[64:128, :])
        nc.vector.tensor_copy(v_sb[:, 4 * b : 4 * b + 4, 0:H], tp)
        yield

    def att(b, nxt=None, out_g=None):
        blk = slice(512 * b, 512 * (b + 1))
        # P@V accumulates into two independent 256-column groups living in
        # separate PSUM banks (full-bank-shaped tiles force the separation;
        # two accumulation groups must never share a bank). Group 0 covers
        # cols 0:256 and stops at chunk 4b+1, so its normalization chain runs
        # ~2 chunks early and only group 1's tail is exposed at the end.
        if out_g is None:
            out_g = [o_ps.tile([65, 512], F32, tag="oa", name=f"oa{b}"),
                     o_ps.tile([65, 512], F32, tag="ob", name=f"ob{b}")]
        fin = fin_pool.tile([64, 512], F32)
        nj = 4 * b + 4
        pts = {}
        _dbg_pts = []

        def tail_group(g):
            cols = slice(256 * g, 256 * (g + 1))
            rc = rc_pool.tile([1, 256], F32, tag="rc", name=f"rc{b}_{g}")
            nc.vector.reciprocal(rc, out_g[g][64:65, 0:256])
            rb = rb_pool.tile([64, 256], F32, tag="rb", name=f"rb{b}_{g}")
            nc.gpsimd.partition_broadcast(rb, rc, channels=64)
            nc.vector.tensor_mul(fin[:, cols], out_g[g][0:64, 0:256], rb)
            if g == 1:
                nc.sync.dma_start(outT[:, blk], fin)

        # chunk "units": off-diagonal chunks are paired so each pair shares
        # one [128,1024] ST tile (two PSUM banks, one accumulation group per
        # bank) and a SINGLE exp instruction — the scalar engine's ~190ns
        # fixed cost per activation is what paces the attention phase.
        # Diagonal chunks stay single (their exp widths shrink with r and the
        # zero pads in pt_diag must not be overwritten).
        units = [(2 * u, 2 * u + 1) for u in range(2 * b)]
        units += [(4 * b + r,) for r in range(4)]

        def emit_st_unit(u):
            chunks = units[u]
            j0 = chunks[0]
            r = j0 - 4 * b
            st = st_ps.tile([128, 1024], F32, tag="st", name=f"st{b}_{u}")
            if r < 0:
                pt = pt_pool.tile([128, 1024], BF16, tag="pt", name=f"pt{b}_{u}")
                for i, j in enumerate(chunks):
                    nc.tensor.matmul(
                        st[:, 512 * i : 512 * i + 512],
                        kv_sb[0:64, 128 * j : 128 * (j + 1)],
                        qT_sb[:, blk], start=True, stop=True)
                nc.scalar.activation(
                    pt, st, func=mybir.ActivationFunctionType.Exp, scale=0.125)
                for i, j in enumerate(chunks):
                    pts[j] = (pt, 512 * i)
                _dbg_pts.append(pt[:, 0:512])
                return
            else:
                coff = 128 * r
                width = 512 - coff
                if r > 0:
                    pt = pt_diag[(b % 2, r)]
                else:
                    pt = pt_pool.tile([128, 1024], BF16, tag="pt",
                                      name=f"pt{b}_{u}")
                nc.tensor.matmul(st[:, 0:width],
                                 kv_sb[0:64, 128 * j0 : 128 * (j0 + 1)],
                                 qT_sb[:, 512 * b + coff : 512 * (b + 1)],
                                 start=True, stop=True)
                nc.vector.tensor_add(st[:, 0:128], st[:, 0:128], neg_mask)
                nc.scalar.activation(
                    pt[:, coff:512], st[:, 0:width],
                    func=mybir.ActivationFunctionType.Exp, scale=0.125)
                pts[j0] = (pt, 0)
            _dbg_pts.append(pt[:, 0:512])

        def emit_pv_unit(u):
            for j in units[u]:
                r = j - 4 * b
                pt, f = pts.pop(j)
                for g in range(2):
                    if r > 2 * g + 1:
                        continue
                    jstop = 4 * b + 2 * g + 1
                    nc.tensor.matmul(out_g[g][:, 0:256], v_sb[:, j, :],
                                     pt[:, f + 256 * g : f + 256 * (g + 1)],
                                     start=(j == 0), stop=(j == jstop))
                    if j == jstop:
                        tail_group(g)

        # software-pipeline the emission so the PE never head-blocks on an
        # exp that hasn't finished (PV for unit u emitted after ST of unit
        # u+2), and drain the next block's projection pieces into the unit
        # slots; the drain is skewed toward later slots because the x DMA for
        # block b+1 is still in flight during the early slots
        NPC = 9
        nu = len(units)
        LAG = CONFIG["pv_lag"]
        targets = CONFIG["drain"][nu] if nxt is not None else None
        drained = 0
        for u in range(nu):
            emit_st_unit(u)
            if nxt is not None:
                while drained < targets[u]:
                    next(nxt, None)
                    drained += 1
            if u >= LAG:
                emit_pv_unit(u - LAG)
            yield
        if nxt is not None:
            while drained < NPC:
                next(nxt, None)
                drained += 1
        for u in range(max(nu - LAG, 0), nu):
            emit_pv_unit(u)
            yield

        if DEBUG_ATT and b == 0:
            dbg = const.tile([65, 512], F32)
            nc.vector.tensor_copy(dbg[:, 0:256], out_g[0][:, 0:256])
            nc.vector.tensor_copy(dbg[:, 256:512], out_g[1][:, 0:256])
            _DBG["out_t0"] = dbg
            for j, p in enumerate(_dbg_pts):
                nc.sync.dma_start(_DBG["d_pt_aps"][j], p)

    gens = [gen_proj(b) for b in range(NB)]
    for _ in gens[0]:
        pass
    for _ in att(0, gens[1]):
        pass
    for _ in att(1, gens[2]):
        pass
    # att(2) is PE-heavy (it carries proj(3)'s matmuls) while att(3) is
    # exp-heavy with nothing left to fill the PE; interleaving their unit
    # streams balances both engines. att(3)'s PV accumulators borrow the
    # projection pools' PSUM banks, which are dead after proj(3)'s copies.
    a2 = att(2, gens[3])
    for _ in range(CONFIG["s2_solo"]):
        next(a2, None)
    out3 = [qk_ps.tile([65, 512], F32, tag="q", name="o3a"),
            kv_ps.tile([65, 512], F32, tag="kv", name="o3b")]
    a3 = att(3, None, out_g=out3)
    done2 = done3 = False
    while not (done2 and done3):
        if not done3:
            done3 = next(a3, StopIteration) is StopIteration
        if not done2:
            done2 = next(a2, StopIteration) is StopIteration

    # debug hook: stash persistent tiles so a debug build can dump them
    _DBG.update({"qT_sb": qT_sb, "kv_sb": kv_sb, "v_sb": v_sb, "xt": xt})


_DBG = {}
DEBUG_ATT = False


_NC = None


def _get_nc():
    global _NC
    if _NC is None:
        _NC = build_bass()
    return _NC


def _pack_w(Wq, Wk, Wv, npbf):
    def chunks(W):
        return np.ascontiguousarray(W.reshape(NCH, 128, H).transpose(1, 0, 2))
    wq = np.ascontiguousarray(chunks(Wq)).astype(npbf)
    wkv = np.ascontiguousarray(
        np.concatenate([chunks(Wk), chunks(Wv)], axis=2)).astype(npbf)
    return wq, wkv


def kernel(x, Wq, Wk, Wv):
    nc = _get_nc()
    npbf = mybir.dt.np(BF16)
    wq, wkv = _pack_w(Wq, Wk, Wv, npbf)
    in_maps = []
    for b in range(8):
        in_maps.append({
            "xT": np.ascontiguousarray(x[b].T).astype(npbf),
            "wq": wq,
            "wkv": wkv,
        })
    res = bass_utils.run_bass_kernel_spmd(nc, in_maps, core_ids=list(range(8)))
    out = np.stack([np.ascontiguousarray(res.results[b]["outT"].T)
                    for b in range(8)])
    return out.astype(np.float32)


# revision 68
# speedup vs baseline: 1.1224x; 1.0452x over previous
"""Single-head causal attention kernel for Trainium2 (8 NeuronCores).

Problem: x[8, 2048, 1024], Wq/Wk/Wv[1024, 64] ->
  out[b] = softmax(causal((x[b] @ Wq) @ (x[b] @ Wk)^T / 8)) @ (x[b] @ Wv)

Sharding: data-parallel over batch, one batch element per core, weights
replicated.

v2 design (vs v1 baseline at 62.8us):
  - x and weights are converted to bf16 on the host (matmul rate is the same
    as f32r but DMA bytes halve; accumulation stays f32 in PSUM; measured
    rel-err ~2e-3 vs the 2e-2 gate)
  - host pre-packs [Wq|Wk|Wv] into one [128, 8, 192] chunk-major tensor so
    the weight load is a single 128x3KB-descriptor DMA (v1 used 256B
    descriptors which pay a 2x small-transfer penalty)
  - x is loaded with 8 large DMAs (block-major), front-loaded in consumption
    order; block 0 is split into 2-chunk pieces so proj(0) can start early
  - per t-block: q chain [64,512] + [Wk|Wv] chain [128,512]; k and v land in
    one [128,T] sbuf tile with a single PSUM->SBUF copy (k rows 0:64 base 0
    for the score matmul, v rows 64:128 base 64 for the PE transpose)
  - scores computed transposed: ST_j = kT_j^T . qT, diag tile masked
    additively, exp on the scalar engine into bf16 PT
  - P@V is split into four 128-column accumulation groups per block, so the
    contraction skips chunks with j > 4b+g (17408 PE rows instead of 20480)
    and no pt zero-padding is needed
  - softmax denominator comes from an appended ones column in v (row 64 of
    the PV output); normalization = DVE reciprocal + Pool-engine
    partition_broadcast + DVE multiply (v1 used a PE broadcast matmul)
"""

import numpy as np
from contextlib import ExitStack

import concourse.bass as bass
import concourse.tile as tile
import concourse.bacc as bacc
from concourse import mybir
from concourse import bass_utils
from concourse.masks import make_identity

F32 = mybir.dt.float32
BF16 = mybir.dt.bfloat16

T = 2048
C = 1024
H = 64
NCH = C // 128   # 8 contraction chunks
NB = T // 512    # 4 t-blocks
NEG = -1.0e9

# schedule knobs (tuned against TimelineSim)
CONFIG = {
    # x DMA stream: list of (block, chunk_lo, chunk_hi) in issue order
    "x_pieces": [(0, 0, 2), (0, 2, 4), (0, 4, 6), (0, 6, 8),
                 (1, 0, 2), (1, 2, 4), (1, 4, 6), (1, 6, 8),
                 (2, 0, 8), (3, 0, 8)],
    # per-unit cumulative drain targets of the next block's 9 proj pieces,
    # keyed by number of units in the current block
    "drain": {4: [2, 4, 7, 9],
              8: [2, 4, 5, 6, 7, 8, 9, 9],
              12: [2, 3, 4, 5, 6, 7, 8, 9, 9, 9, 9, 9],
              16: [2, 3, 4, 5, 6, 7, 8, 9, 9, 9, 9, 9, 9, 9, 9, 9]},
    # pair off-diagonal chunks into one exp (2-bank ST tiles, depth 2)
    # vs single-chunk STs (1-bank tiles, depth 4)
    "pair": False,
    # PV emission lag in units
    "pv_lag": 7,
    # solo steps of att(2) before att(3) interleaving begins; must be large
    # enough that proj(3) is fully drained first
    "s2_solo": 8,
    # PE p-state warmup: dependency-free matmuls at the start (the ramp to
    # 2.4GHz needs 3us of continuous busy; x DMA keeps the PE idle until
    # ~3.9us otherwise), and a bridge over the x1-wait after att(0)
    "warm_head": 0,
    "bridge01": 0,
}


def build_bass():
    nc = bacc.Bacc("TRN2", target_bir_lowering=False, debug=False, num_devices=8)
    xT = nc.dram_tensor("xT", [C, T], BF16, kind="ExternalInput").ap()
    wq_d = nc.dram_tensor("wq", [128, NCH, 64], BF16, kind="ExternalInput").ap()
    wkv_d = nc.dram_tensor("wkv", [128, NCH, 128], BF16,
                           kind="ExternalInput").ap()
    outT = nc.dram_tensor("outT", [H, T], F32, kind="ExternalOutput").ap()

    with tile.TileContext(nc) as tc:
        with ExitStack() as ctx:
            build_kernel(ctx, tc, nc, xT, (wq_d, wkv_d), outT)
    nc.compile()
    return nc


def build_kernel(ctx, tc, nc, xT, wdrams, outT):
    wq_d, wkv_d = wdrams
    const = ctx.enter_context(tc.tile_pool(name="const", bufs=1))
    pt_pool = ctx.enter_context(
        tc.tile_pool(name="pt", bufs=CONFIG["pv_lag"] + 2))
    fin_pool = ctx.enter_context(tc.tile_pool(name="fin", bufs=2))
    rc_pool = ctx.enter_context(tc.tile_pool(name="rc", bufs=2))
    rb_pool = ctx.enter_context(tc.tile_pool(name="rb", bufs=2))
    qk_ps = ctx.enter_context(tc.tile_pool(name="qkps", bufs=1, space="PSUM"))
    kv_ps = ctx.enter_context(tc.tile_pool(name="kvps", bufs=1, space="PSUM"))
    st_ps = ctx.enter_context(
        tc.tile_pool(name="stps", bufs=2 if CONFIG["pair"] else 4,
                     space="PSUM"))
    o_ps = ctx.enter_context(tc.tile_pool(name="ops", bufs=1, space="PSUM"))

    # persistent sbuf state
    xt = const.tile([128, NB, NCH, 512], BF16)   # x^T, block-major chunks
    w_q = const.tile([128, NCH, 64], BF16)       # Wq per c-chunk
    w_kv = const.tile([128, NCH, 128], BF16)     # [Wk|Wv] per c-chunk
    qT_sb = const.tile([64, T], BF16)
    kv_sb = const.tile([128, T], BF16)           # rows 0:64 kT, 64:128 vT
    v_sb = const.tile([128, T // 128, H + 1], BF16)  # v natural + ones col
    neg_mask = const.tile([128, 128], F32)       # 0 where t>=s, -1e9 below
    idb = const.tile([128, 64], BF16)            # identity in rows 64:128

    # weights then x, in consumption order; block 0 in small pieces so the
    # first projection chain can start as soon as possible
    # wq first, then the first x piece, then wkv: the first q-chain matmul
    # only needs wq + x chunk 0, so it starts one transfer earlier
    nc.sync.dma_start(w_q, wq_d)
    xr = xT.rearrange("(j p) t -> p j t", p=128)
    for i, (blkno, lo, hi) in enumerate(CONFIG["x_pieces"]):
        blk = slice(512 * blkno, 512 * (blkno + 1))
        nc.sync.dma_start(xt[:, blkno, lo:hi, :], xr[:, lo:hi, blk])
        if i == 0:
            nc.sync.dma_start(w_kv, wkv_d)

    # constants
    nc.gpsimd.memset(neg_mask, 0.0)
    nc.gpsimd.affine_select(
        out=neg_mask, in_=neg_mask, compare_op=mybir.AluOpType.is_ge,
        fill=NEG, base=0, pattern=[[1, 128]], channel_multiplier=-1,
    )
    id_f = const.tile([128, 64], F32)
    make_identity(nc, id_f[64:128, :])
    nc.vector.tensor_copy(idb[64:128, :], id_f[64:128, :])
    ones_f = const.tile([128, 16, 1], F32)
    nc.vector.memset(ones_f, 1.0)
    nc.vector.tensor_copy(v_sb[:, :, H : H + 1], ones_f)
    # dedicated PT slots for diagonal s-chunks, one per within-block offset r:
    # the pad region [0:128r] is zeroed once here and never overwritten (exp
    # always writes exactly [128r:512]), so the full-width P@V matmul reads
    # zeros above the diagonal
    # two sets so adjacent blocks can be processed interleaved without
    # write-after-read fences on the shared diag slots
    pt_diag = {(s, r): const.tile([128, 512], BF16, name=f"pt_diag{s}_{r}")
               for s in range(2) for r in range(1, 4)}
    zero_f = const.tile([128, 384], F32)
    nc.vector.memset(zero_f, 0.0)
    for s in range(2):
        for r in range(1, 4):
            nc.vector.tensor_copy(pt_diag[(s, r)][:, 0 : 128 * r],
                                  zero_f[:, 0 : 128 * r])

    # junk operand for p-state warmup / gap-bridging matmuls
    warm_sb = const.tile([128, 512], BF16)
    nc.vector.memset(warm_sb, 1.0)

    def emit_warm(n, pool, tag, name):
        """Dependency-free junk matmuls that keep the PE ramp clock running
        through an unavoidable stall. Rotating 128-col targets avoid WAW
        chains so they stream back-to-back."""
        if n <= 0:
            return
        wps = pool.tile([128, 512], F32, tag=tag, name=name)
        for k in range(n):
            c = 128 * (k % 4)
            nc.tensor.matmul(wps[:, c : c + 128], warm_sb[:, 0:128],
                             warm_sb[:, 0:128], start=True, stop=True,
                             skip_group_check=True)

    # head warmup: runs while the first x pieces stream in, so the ramp
    # clock is already past 3us when the first projection matmul issues
    emit_warm(CONFIG["warm_head"], qk_ps, "q", "warm0")

    def gen_proj(b):
        """Projection for block b as a generator of 9 'pieces' so it can be
        interleaved into the previous block's attention chunk loop (keeps the
        PE fed while the scalar engine works through the exps)."""
        blk = slice(512 * b, 512 * (b + 1))
        q_t = qk_ps.tile([64, 512], F32, tag="q")
        kv_t = kv_ps.tile([128, 512], F32, tag="kv")
        for j in range(NCH):
            nc.tensor.matmul(q_t, w_q[:, j, :], xt[:, b, j, :],
                             start=(j == 0), stop=(j == NCH - 1))
            nc.tensor.matmul(kv_t, w_kv[:, j, :], xt[:, b, j, :],
                             start=(j == 0), stop=(j == NCH - 1))
            yield
        nc.vector.tensor_copy(qT_sb[:, blk], q_t)
        nc.vector.tensor_copy(kv_sb[:, blk], kv_t)
        # v natural layout via PE transpose; vT sits at rows 64:128 so the
        # transpose operands (vT slice, identity rows 64:128) share
        # base_partition
        tp = qk_ps.tile([128, 4, 64], BF16, tag="q", name=f"tp{b}")
        for r in range(4):
            nc.tensor.transpose(
                tp[:, r, :],
                kv_sb[64:128, 512 * b + 128 * r : 512 * b + 128 * (r + 1)],
                idb[64:128, :])
        nc.vector.tensor_copy(v_sb[:, 4 * b : 4 * b + 4, 0:H], tp)
        yield

    def att(b, nxt=None, out_g=None):
        blk = slice(512 * b, 512 * (b + 1))
        # P@V accumulates into two independent 256-column groups living in
        # separate PSUM banks (full-bank-shaped tiles force the separation;
        # two accumulation groups must never share a bank). Group 0 covers
        # cols 0:256 and stops at chunk 4b+1, so its normalization chain runs
        # ~2 chunks early and only group 1's tail is exposed at the end.
        if out_g is None:
            out_g = [o_ps.tile([65, 512], F32, tag="oa", name=f"oa{b}"),
                     o_ps.tile([65, 512], F32, tag="ob", name=f"ob{b}")]
        fin = fin_pool.tile([64, 512], F32)
        nj = 4 * b + 4
        pts = {}
        _dbg_pts = []

        def tail_group(g):
            cols = slice(256 * g, 256 * (g + 1))
            rc = rc_pool.tile([1, 256], F32, tag="rc", name=f"rc{b}_{g}")
            nc.vector.reciprocal(rc, out_g[g][64:65, 0:256])
            rb = rb_pool.tile([64, 256], F32, tag="rb", name=f"rb{b}_{g}")
            nc.gpsimd.partition_broadcast(rb, rc, channels=64)
            nc.vector.tensor_mul(fin[:, cols], out_g[g][0:64, 0:256], rb)
            if g == 1:
                nc.sync.dma_start(outT[:, blk], fin)

        # chunk "units": off-diagonal chunks are paired so each pair shares
        # one [128,1024] ST tile (two PSUM banks, one accumulation group per
        # bank) and a SINGLE exp instruction — the scalar engine's ~190ns
        # fixed cost per activation is what paces the attention phase.
        # Diagonal chunks stay single (their exp widths shrink with r and the
        # zero pads in pt_diag must not be overwritten).
        if CONFIG["pair"]:
            units = [(2 * u, 2 * u + 1) for u in range(2 * b)]
        else:
            units = [(j,) for j in range(4 * b)]
        units += [(4 * b + r,) for r in range(4)]

        def emit_st_unit(u):
            chunks = units[u]
            j0 = chunks[0]
            r = j0 - 4 * b
            stw = 512 * len(chunks)
            st = st_ps.tile([128, stw], F32, tag="st", name=f"st{b}_{u}")
            if r < 0:
                pt = pt_pool.tile([128, stw], BF16, tag="pt", name=f"pt{b}_{u}")
                for i, j in enumerate(chunks):
                    nc.tensor.matmul(
                        st[:, 512 * i : 512 * i + 512],
                        kv_sb[0:64, 128 * j : 128 * (j + 1)],
                        qT_sb[:, blk], start=True, stop=True)
                nc.scalar.activation(
                    pt, st, func=mybir.ActivationFunctionType.Exp, scale=0.125)
                for i, j in enumerate(chunks):
                    pts[j] = (pt, 512 * i)
                _dbg_pts.append(pt[:, 0:512])
                return
            else:
                coff = 128 * r
                width = 512 - coff
                if r > 0:
                    pt = pt_diag[(b % 2, r)]
                else:
                    pt = pt_pool.tile([128, 512], BF16, tag="pt",
                                      name=f"pt{b}_{u}")
                nc.tensor.matmul(st[:, 0:width],
                                 kv_sb[0:64, 128 * j0 : 128 * (j0 + 1)],
                                 qT_sb[:, 512 * b + coff : 512 * (b + 1)],
                                 start=True, stop=True)
                nc.vector.tensor_add(st[:, 0:128], st[:, 0:128], neg_mask)
                nc.scalar.activation(
                    pt[:, coff:512], st[:, 0:width],
                    func=mybir.ActivationFunctionType.Exp, scale=0.125)
                pts[j0] = (pt, 0)
            _dbg_pts.append(pt[:, 0:512])

        def emit_pv_unit(u):
            for j in units[u]:
                r = j - 4 * b
                pt, f = pts.pop(j)
                for g in range(2):
                    if r > 2 * g + 1:
                        continue
                    jstop = 4 * b + 2 * g + 1
                    nc.tensor.matmul(out_g[g][:, 0:256], v_sb[:, j, :],
                                     pt[:, f + 256 * g : f + 256 * (g + 1)],
                                     start=(j == 0), stop=(j == jstop))
                    if j == jstop:
                        tail_group(g)

        # software-pipeline the emission so the PE never head-blocks on an
        # exp that hasn't finished (PV for unit u emitted after ST of unit
        # u+2), and drain the next block's projection pieces into the unit
        # slots; the drain is skewed toward later slots because the x DMA for
        # block b+1 is still in flight during the early slots
        NPC = 9
        nu = len(units)
        LAG = CONFIG["pv_lag"]
        targets = CONFIG["drain"][nu] if nxt is not None else None
        drained = 0
        for u in range(nu):
            emit_st_unit(u)
            if nxt is not None:
                while drained < targets[u]:
                    next(nxt, None)
                    drained += 1
            if u >= LAG:
                emit_pv_unit(u - LAG)
            yield
        if nxt is not None:
            while drained < NPC:
                next(nxt, None)
                drained += 1
        for u in range(max(nu - LAG, 0), nu):
            emit_pv_unit(u)
            yield

        if DEBUG_ATT and b == 0:
            dbg = const.tile([65, 512], F32)
            nc.vector.tensor_copy(dbg[:, 0:256], out_g[0][:, 0:256])
            nc.vector.tensor_copy(dbg[:, 256:512], out_g[1][:, 0:256])
            _DBG["out_t0"] = dbg
            for j, p in enumerate(_dbg_pts):
                nc.sync.dma_start(_DBG["d_pt_aps"][j], p)

    gens = [gen_proj(b) for b in range(NB)]
    for _ in gens[0]:
        pass
    for _ in att(0, gens[1]):
        pass
    # bridge the x1-transfer wait so the PE ramp clock keeps running
    emit_warm(CONFIG["bridge01"], st_ps, "st", "warm1")
    for _ in att(1, gens[2]):
        pass
    # att(2) is PE-heavy (it carries proj(3)'s matmuls) while att(3) is
    # exp-heavy with nothing left to fill the PE; interleaving their unit
    # streams balances both engines. att(3)'s PV accumulators borrow the
    # projection pools' PSUM banks, which are dead after proj(3)'s copies.
    a2 = att(2, gens[3])
    for _ in range(CONFIG["s2_solo"]):
        next(a2, None)
    out3 = [qk_ps.tile([65, 512], F32, tag="q", name="o3a"),
            kv_ps.tile([65, 512], F32, tag="kv", name="o3b")]
    a3 = att(3, None, out_g=out3)
    done2 = done3 = False
    while not (done2 and done3):
        if not done3:
            done3 = next(a3, StopIteration) is StopIteration
        if not done2:
            done2 = next(a2, StopIteration) is StopIteration

    # debug hook: stash persistent tiles so a debug build can dump them
    _DBG.update({"qT_sb": qT_sb, "kv_sb": kv_sb, "v_sb": v_sb, "xt": xt})


_DBG = {}
DEBUG_ATT = False


_NC = None


def _get_nc():
    global _NC
    if _NC is None:
        _NC = build_bass()
    return _NC


def _pack_w(Wq, Wk, Wv, npbf):
    def chunks(W):
        return np.ascontiguousarray(W.reshape(NCH, 128, H).transpose(1, 0, 2))
    wq = np.ascontiguousarray(chunks(Wq)).astype(npbf)
    wkv = np.ascontiguousarray(
        np.concatenate([chunks(Wk), chunks(Wv)], axis=2)).astype(npbf)
    return wq, wkv


def kernel(x, Wq, Wk, Wv):
    nc = _get_nc()
    npbf = mybir.dt.np(BF16)
    wq, wkv = _pack_w(Wq, Wk, Wv, npbf)
    in_maps = []
    for b in range(8):
        in_maps.append({
            "xT": np.ascontiguousarray(x[b].T).astype(npbf),
            "wq": wq,
            "wkv": wkv,
        })
    res = bass_utils.run_bass_kernel_spmd(nc, in_maps, core_ids=list(range(8)))
    out = np.stack([np.ascontiguousarray(res.results[b]["outT"].T)
                    for b in range(8)])
    return out.astype(np.float32)


# revision 87
# speedup vs baseline: 1.1267x; 1.0038x over previous
"""Single-head causal attention kernel for Trainium2 (8 NeuronCores).

Problem: x[8, 2048, 1024], Wq/Wk/Wv[1024, 64] ->
  out[b] = softmax(causal((x[b] @ Wq) @ (x[b] @ Wk)^T / 8)) @ (x[b] @ Wv)

Sharding: data-parallel over batch, one batch element per core, weights
replicated.

v2 design (vs v1 baseline at 62.8us):
  - x and weights are converted to bf16 on the host (matmul rate is the same
    as f32r but DMA bytes halve; accumulation stays f32 in PSUM; measured
    rel-err ~2e-3 vs the 2e-2 gate)
  - host pre-packs [Wq|Wk|Wv] into one [128, 8, 192] chunk-major tensor so
    the weight load is a single 128x3KB-descriptor DMA (v1 used 256B
    descriptors which pay a 2x small-transfer penalty)
  - x is loaded with 8 large DMAs (block-major), front-loaded in consumption
    order; block 0 is split into 2-chunk pieces so proj(0) can start early
  - per t-block: q chain [64,512] + [Wk|Wv] chain [128,512]; k and v land in
    one [128,T] sbuf tile with a single PSUM->SBUF copy (k rows 0:64 base 0
    for the score matmul, v rows 64:128 base 64 for the PE transpose)
  - scores computed transposed: ST_j = kT_j^T . qT, diag tile masked
    additively, exp on the scalar engine into bf16 PT
  - P@V is split into four 128-column accumulation groups per block, so the
    contraction skips chunks with j > 4b+g (17408 PE rows instead of 20480)
    and no pt zero-padding is needed
  - softmax denominator comes from an appended ones column in v (row 64 of
    the PV output); normalization = DVE reciprocal + Pool-engine
    partition_broadcast + DVE multiply (v1 used a PE broadcast matmul)
"""

import numpy as np
from contextlib import ExitStack

import concourse.bass as bass
import concourse.tile as tile
import concourse.bacc as bacc
from concourse import mybir
from concourse import bass_utils
from concourse.masks import make_identity

F32 = mybir.dt.float32
BF16 = mybir.dt.bfloat16

T = 2048
C = 1024
H = 64
NCH = C // 128   # 8 contraction chunks
NB = T // 512    # 4 t-blocks
NEG = -1.0e9

# schedule knobs (tuned against TimelineSim)
CONFIG = {
    # x DMA stream: list of (block, chunk_lo, chunk_hi) in issue order
    "x_pieces": [(0, 0, 2), (0, 2, 4), (0, 4, 6), (0, 6, 8),
                 (1, 0, 2), (1, 2, 4), (1, 4, 6), (1, 6, 8),
                 (2, 0, 8), (3, 0, 8)],
    # per-unit cumulative drain targets of the next block's 9 proj pieces,
    # keyed by number of units in the current block
    "drain": {4: [2, 4, 6, 9],
              8: [2, 4, 5, 6, 7, 8, 9, 9],
              12: [2, 3, 4, 5, 6, 7, 8, 9, 9, 9, 9, 9],
              16: [2, 3, 4, 5, 6, 7, 8, 9, 9, 9, 9, 9, 9, 9, 9, 9]},
    # blocks whose off-diagonal chunks are paired into one exp (2-bank ST
    # tiles, pool depth 2) — the late blocks are Act-paced with no proj work
    # left to fill the PE, so halving the per-exp overhead helps there;
    # early blocks keep single-chunk STs (1-bank tiles, pool depth 4... 2
    # when any block pairs, since pool slots are sized for the largest tile)
    "pair_blocks": (),
    # PV emission lag in units
    "pv_lag": 7,
    # solo steps of att(2) before att(3) interleaving begins; must be large
    # enough that proj(3) is fully drained first
    "s2_solo": 8,
    # PE p-state warmup: dependency-free matmuls at the start (the ramp to
    # 2.4GHz needs 3us of continuous busy; x DMA keeps the PE idle until
    # ~3.9us otherwise), and a bridge over the x1-wait after att(0)
    # separate out-DMA per 256-col group (group 0 fires early)
    "split_fin": False,
    "warm_head": 0,
    "bridge01": 0,
}


def build_bass():
    nc = bacc.Bacc("TRN2", target_bir_lowering=False, debug=False, num_devices=8)
    xT = nc.dram_tensor("xT", [C, T], BF16, kind="ExternalInput").ap()
    wq_d = nc.dram_tensor("wq", [128, NCH, 64], BF16, kind="ExternalInput").ap()
    wkv_d = nc.dram_tensor("wkv", [128, NCH, 128], BF16,
                           kind="ExternalInput").ap()
    outT = nc.dram_tensor("outT", [H, T], F32, kind="ExternalOutput").ap()

    with tile.TileContext(nc) as tc:
        with ExitStack() as ctx:
            build_kernel(ctx, tc, nc, xT, (wq_d, wkv_d), outT)
    nc.compile()
    return nc


def build_kernel(ctx, tc, nc, xT, wdrams, outT):
    wq_d, wkv_d = wdrams
    const = ctx.enter_context(tc.tile_pool(name="const", bufs=1))
    pt_pool = ctx.enter_context(
        tc.tile_pool(name="pt", bufs=CONFIG["pv_lag"] + 2))
    fin_pool = ctx.enter_context(tc.tile_pool(name="fin", bufs=2))
    rc_pool = ctx.enter_context(tc.tile_pool(name="rc", bufs=2))
    rb_pool = ctx.enter_context(tc.tile_pool(name="rb", bufs=2))
    qk_ps = ctx.enter_context(tc.tile_pool(name="qkps", bufs=1, space="PSUM"))
    kv_ps = ctx.enter_context(tc.tile_pool(name="kvps", bufs=1, space="PSUM"))
    st_ps = ctx.enter_context(
        tc.tile_pool(name="stps", bufs=2 if CONFIG["pair_blocks"] else 4,
                     space="PSUM"))
    o_ps = ctx.enter_context(tc.tile_pool(name="ops", bufs=1, space="PSUM"))

    # persistent sbuf state
    xt = const.tile([128, NB, NCH, 512], BF16)   # x^T, block-major chunks
    w_q = const.tile([128, NCH, 64], BF16)       # Wq per c-chunk
    w_kv = const.tile([128, NCH, 128], BF16)     # [Wk|Wv] per c-chunk
    qT_sb = const.tile([64, T], BF16)
    kv_sb = const.tile([128, T], BF16)           # rows 0:64 kT, 64:128 vT
    v_sb = const.tile([128, T // 128, H + 1], BF16)  # v natural + ones col
    neg_mask = const.tile([128, 128], F32)       # 0 where t>=s, -1e9 below
    idb = const.tile([128, 64], BF16)            # identity in rows 64:128

    # wq first, then the first x piece, then wkv: the first q-chain matmul
    # only needs wq + x chunk 0, so it starts one transfer earlier
    nc.sync.dma_start(w_q, wq_d)
    xr = xT.rearrange("(j p) t -> p j t", p=128)
    for i, (blkno, lo, hi) in enumerate(CONFIG["x_pieces"]):
        blk = slice(512 * blkno, 512 * (blkno + 1))
        nc.sync.dma_start(xt[:, blkno, lo:hi, :], xr[:, lo:hi, blk])
        if i == 0:
            nc.sync.dma_start(w_kv, wkv_d)

    # constants
    nc.gpsimd.memset(neg_mask, 0.0)
    nc.gpsimd.affine_select(
        out=neg_mask, in_=neg_mask, compare_op=mybir.AluOpType.is_ge,
        fill=NEG, base=0, pattern=[[1, 128]], channel_multiplier=-1,
    )
    id_f = const.tile([128, 64], F32)
    make_identity(nc, id_f[64:128, :])
    nc.vector.tensor_copy(idb[64:128, :], id_f[64:128, :])
    ones_f = const.tile([128, 16, 1], F32)
    nc.vector.memset(ones_f, 1.0)
    nc.vector.tensor_copy(v_sb[:, :, H : H + 1], ones_f)
    # dedicated PT slots for diagonal s-chunks, one per within-block offset r:
    # the pad region [0:128r] is zeroed once here and never overwritten (exp
    # always writes exactly [128r:512]), so the full-width P@V matmul reads
    # zeros above the diagonal
    # two sets so adjacent blocks can be processed interleaved without
    # write-after-read fences on the shared diag slots
    pt_diag = {(s, r): const.tile([128, 512], BF16, name=f"pt_diag{s}_{r}")
               for s in range(2) for r in range(1, 4)}
    zero_f = const.tile([128, 384], F32)
    nc.vector.memset(zero_f, 0.0)
    for s in range(2):
        for r in range(1, 4):
            nc.vector.tensor_copy(pt_diag[(s, r)][:, 0 : 128 * r],
                                  zero_f[:, 0 : 128 * r])

    # junk operand for p-state warmup / gap-bridging matmuls
    warm_sb = const.tile([128, 512], BF16)
    nc.vector.memset(warm_sb, 1.0)

    def emit_warm(n, pool, tag, name):
        """Dependency-free junk matmuls that keep the PE ramp clock running
        through an unavoidable stall. Rotating 128-col targets avoid WAW
        chains so they stream back-to-back."""
        if n <= 0:
            return
        wps = pool.tile([128, 512], F32, tag=tag, name=name)
        for k in range(n):
            c = 128 * (k % 4)
            nc.tensor.matmul(wps[:, c : c + 128], warm_sb[:, 0:128],
                             warm_sb[:, 0:128], start=True, stop=True,
                             skip_group_check=True)

    # head warmup: runs while the first x pieces stream in, so the ramp
    # clock is already past 3us when the first projection matmul issues
    emit_warm(CONFIG["warm_head"], qk_ps, "q", "warm0")

    def gen_proj(b):
        """Projection for block b as a generator of 9 'pieces' so it can be
        interleaved into the previous block's attention chunk loop (keeps the
        PE fed while the scalar engine works through the exps)."""
        blk = slice(512 * b, 512 * (b + 1))
        q_t = qk_ps.tile([64, 512], F32, tag="q")
        kv_t = kv_ps.tile([128, 512], F32, tag="kv")
        for j in range(NCH):
            nc.tensor.matmul(q_t, w_q[:, j, :], xt[:, b, j, :],
                             start=(j == 0), stop=(j == NCH - 1))
            nc.tensor.matmul(kv_t, w_kv[:, j, :], xt[:, b, j, :],
                             start=(j == 0), stop=(j == NCH - 1))
            yield
        nc.vector.tensor_copy(qT_sb[:, blk], q_t)
        nc.vector.tensor_copy(kv_sb[:, blk], kv_t)
        # v natural layout via PE transpose; vT sits at rows 64:128 so the
        # transpose operands (vT slice, identity rows 64:128) share
        # base_partition
        tp = qk_ps.tile([128, 4, 64], BF16, tag="q", name=f"tp{b}")
        for r in range(4):
            nc.tensor.transpose(
                tp[:, r, :],
                kv_sb[64:128, 512 * b + 128 * r : 512 * b + 128 * (r + 1)],
                idb[64:128, :])
        nc.vector.tensor_copy(v_sb[:, 4 * b : 4 * b + 4, 0:H], tp)
        yield

    def att(b, nxt=None, out_g=None):
        blk = slice(512 * b, 512 * (b + 1))
        # P@V accumulates into two independent 256-column groups living in
        # separate PSUM banks (full-bank-shaped tiles force the separation;
        # two accumulation groups must never share a bank). Group 0 covers
        # cols 0:256 and stops at chunk 4b+1, so its normalization chain runs
        # ~2 chunks early and only group 1's tail is exposed at the end.
        if out_g is None:
            out_g = [o_ps.tile([65, 512], F32, tag="oa", name=f"oa{b}"),
                     o_ps.tile([65, 512], F32, tag="ob", name=f"ob{b}")]
        fin = fin_pool.tile([64, 512], F32)
        nj = 4 * b + 4
        pts = {}
        _dbg_pts = []

        def tail_group(g):
            cols = slice(256 * g, 256 * (g + 1))
            rc = rc_pool.tile([1, 256], F32, tag="rc", name=f"rc{b}_{g}")
            nc.vector.reciprocal(rc, out_g[g][64:65, 0:256])
            rb = rb_pool.tile([64, 256], F32, tag="rb", name=f"rb{b}_{g}")
            nc.gpsimd.partition_broadcast(rb, rc, channels=64)
            nc.vector.tensor_mul(fin[:, cols], out_g[g][0:64, 0:256], rb)
            if CONFIG["split_fin"]:
                nc.sync.dma_start(outT[:, 512 * b + 256 * g :
                                       512 * b + 256 * (g + 1)], fin[:, cols])
            elif g == 1:
                nc.sync.dma_start(outT[:, blk], fin)

        # chunk "units": off-diagonal chunks are paired so each pair shares
        # one [128,1024] ST tile (two PSUM banks, one accumulation group per
        # bank) and a SINGLE exp instruction — the scalar engine's ~190ns
        # fixed cost per activation is what paces the attention phase.
        # Diagonal chunks stay single (their exp widths shrink with r and the
        # zero pads in pt_diag must not be overwritten).
        if b in CONFIG["pair_blocks"]:
            units = [(2 * u, 2 * u + 1) for u in range(2 * b)]
        else:
            units = [(j,) for j in range(4 * b)]
        units += [(4 * b + r,) for r in range(4)]

        def emit_st_unit(u):
            chunks = units[u]
            j0 = chunks[0]
            r = j0 - 4 * b
            stw = 512 * len(chunks)
            st = st_ps.tile([128, stw], F32, tag="st", name=f"st{b}_{u}")
            if r < 0:
                pt = pt_pool.tile([128, stw], BF16, tag="pt", name=f"pt{b}_{u}")
                for i, j in enumerate(chunks):
                    nc.tensor.matmul(
                        st[:, 512 * i : 512 * i + 512],
                        kv_sb[0:64, 128 * j : 128 * (j + 1)],
                        qT_sb[:, blk], start=True, stop=True)
                nc.scalar.activation(
                    pt, st, func=mybir.ActivationFunctionType.Exp, scale=0.125)
                for i, j in enumerate(chunks):
                    pts[j] = (pt, 512 * i)
                _dbg_pts.append(pt[:, 0:512])
                return
            else:
                coff = 128 * r
                width = 512 - coff
                if r > 0:
                    pt = pt_diag[(b % 2, r)]
                else:
                    pt = pt_pool.tile([128, 512], BF16, tag="pt",
                                      name=f"pt{b}_{u}")
                nc.tensor.matmul(st[:, 0:width],
                                 kv_sb[0:64, 128 * j0 : 128 * (j0 + 1)],
                                 qT_sb[:, 512 * b + coff : 512 * (b + 1)],
                                 start=True, stop=True)
                nc.vector.tensor_add(st[:, 0:128], st[:, 0:128], neg_mask)
                nc.scalar.activation(
                    pt[:, coff:512], st[:, 0:width],
                    func=mybir.ActivationFunctionType.Exp, scale=0.125)
                pts[j0] = (pt, 0)
            _dbg_pts.append(pt[:, 0:512])

        def emit_pv_unit(u):
            for j in units[u]:
                r = j - 4 * b
                pt, f = pts.pop(j)
                for g in range(2):
                    if r > 2 * g + 1:
                        continue
                    jstop = 4 * b + 2 * g + 1
                    nc.tensor.matmul(out_g[g][:, 0:256], v_sb[:, j, :],
                                     pt[:, f + 256 * g : f + 256 * (g + 1)],
                                     start=(j == 0), stop=(j == jstop))
                    if j == jstop:
                        tail_group(g)

        # software-pipeline the emission so the PE never head-blocks on an
        # exp that hasn't finished (PV for unit u emitted after ST of unit
        # u+2), and drain the next block's projection pieces into the unit
        # slots; the drain is skewed toward later slots because the x DMA for
        # block b+1 is still in flight during the early slots
        NPC = 9
        nu = len(units)
        LAG = CONFIG["pv_lag"]
        targets = CONFIG["drain"][nu] if nxt is not None else None
        drained = 0
        for u in range(nu):
            emit_st_unit(u)
            if nxt is not None:
                while drained < targets[u]:
                    next(nxt, None)
                    drained += 1
            if u >= LAG:
                emit_pv_unit(u - LAG)
            yield
        if nxt is not None:
            while drained < NPC:
                next(nxt, None)
                drained += 1
        for u in range(max(nu - LAG, 0), nu):
            emit_pv_unit(u)
            yield

        if DEBUG_ATT and b == 0:
            dbg = const.tile([65, 512], F32)
            nc.vector.tensor_copy(dbg[:, 0:256], out_g[0][:, 0:256])
            nc.vector.tensor_copy(dbg[:, 256:512], out_g[1][:, 0:256])
            _DBG["out_t0"] = dbg
            for j, p in enumerate(_dbg_pts):
                nc.sync.dma_start(_DBG["d_pt_aps"][j], p)

    gens = [gen_proj(b) for b in range(NB)]
    for _ in gens[0]:
        pass
    for _ in att(0, gens[1]):
        pass
    # bridge the x1-transfer wait so the PE ramp clock keeps running
    emit_warm(CONFIG["bridge01"], st_ps, "st", "warm1")
    for _ in att(1, gens[2]):
        pass
    # att(2) is PE-heavy (it carries proj(3)'s matmuls) while att(3) is
    # exp-heavy with nothing left to fill the PE; interleaving their unit
    # streams balances both engines. att(3)'s PV accumulators borrow the
    # projection pools' PSUM banks, which are dead after proj(3)'s copies.
    a2 = att(2, gens[3])
    for _ in range(CONFIG["s2_solo"]):
        next(a2, None)
    out3 = [qk_ps.tile([65, 512], F32, tag="q", name="o3a"),
            kv_ps.tile([65, 512], F32, tag="kv", name="o3b")]
    a3 = att(3, None, out_g=out3)
    done2 = done3 = False
    while not (done2 and done3):
        if not done3:
            done3 = next(a3, StopIteration) is StopIteration
        if not done2:
            done2 = next(a2, StopIteration) is StopIteration

    # debug hook: stash persistent tiles so a debug build can dump them
    _DBG.update({"qT_sb": qT_sb, "kv_sb": kv_sb, "v_sb": v_sb, "xt": xt})


_DBG = {}
DEBUG_ATT = False


_NC = None


def _get_nc():
    global _NC
    if _NC is None:
        _NC = build_bass()
    return _NC


def _pack_w(Wq, Wk, Wv, npbf):
    def chunks(W):
        return np.ascontiguousarray(W.reshape(NCH, 128, H).transpose(1, 0, 2))
    wq = np.ascontiguousarray(chunks(Wq)).astype(npbf)
    wkv = np.ascontiguousarray(
        np.concatenate([chunks(Wk), chunks(Wv)], axis=2)).astype(npbf)
    return wq, wkv


def kernel(x, Wq, Wk, Wv):
    nc = _get_nc()
    npbf = mybir.dt.np(BF16)
    wq, wkv = _pack_w(Wq, Wk, Wv, npbf)
    in_maps = []
    for b in range(8):
        in_maps.append({
            "xT": np.ascontiguousarray(x[b].T).astype(npbf),
            "wq": wq,
            "wkv": wkv,
        })
    res = bass_utils.run_bass_kernel_spmd(nc, in_maps, core_ids=list(range(8)))
    out = np.stack([np.ascontiguousarray(res.results[b]["outT"].T)
                    for b in range(8)])
    return out.astype(np.float32)
